# revision 1
# baseline (speedup 1.0000x reference)
"""Harmonic decomposition kernel for 8 TRN2 NeuronCores.

out[b] = basis^T R(theta_b) (basis @ x_b)   with per-harmonic complex rotation.

Sharding: the N*L*2 = 800 coefficient axis is split into 8 slices of 100
(de-interleaved so real parts sit in rows 0..49 and imaginary parts in rows
50..99 of each slice). Each core projects the full image onto its 100 basis
rows (GEMM1), rotates its coefficients locally, and reconstructs a partial
full-resolution output (GEMM2). The host sums the 8 partials.

Each core's basis slice is shipped in two layouts (ij-major tiles for the
projection's contraction over space, c-major for reconstruction) as float16;
accumulation stays fp32 in PSUM.
"""

import sys

import numpy as np

for _p in ("/opt/trn_rl_repo",):
    if _p not in sys.path:
        sys.path.insert(0, _p)

B = 8          # batch
IJ = 25600     # 160*160 spatial
NL2 = 800      # total coefficients
PC = 100       # coefficients per core
T = 200        # GEMM1 k-tiles (IJ / 128)
P = 128        # partitions
NJ = 50        # GEMM2 n-chunks (IJ / 512)
NCORES = 8


def _patch_tile_drain():
    """This container's walrus caps sem-waits at 1 per instruction; the stock
    Tile tail drain carries one wait per live semaphore. Keep one on the drain
    and emit the rest as individual SP wait instructions before the barrier."""
    import concourse.tile as tile
    from concourse.vector_clock import ScopedClock

    if getattr(tile.TileContext, "_ant_drain_patched", False):
        return

    def _drain_and_barrier(self, tick_clock, wait_clock):
        nc = self.nc
        drain_inst = nc.sync.drain()
        wait_clock.add_sem_waits(
            drain_inst.ins, ScopedClock({None: tick_clock.global_clock})
        )
        si = drain_inst.ins.sync_info
        waits = list(si.on_wait) if si and si.on_wait else []
        if len(waits) > 1:
            num2sem = {s.num: s for s in self.sems.allocated().values()}
            si.on_wait = waits[:1]
            for w in waits[1:]:
                op = {"sem-ge-imm": "sem-ge", "sem-eq-imm": "sem-eq"}[w.wait_mode]
                nc.sync.nop(nofuse=True).wait_op(num2sem[w.id], w.wait_value, op)
        nc.all_engine_barrier()
        assert self.sems is not None
        popped = nc._tile_sem_poison_stack.pop()
        assert popped is self._sem_poison
        nc.clear_and_free_semaphores(list(self.sems.allocated().values()))
        nc.all_engine_barrier()

    tile.TileContext._drain_and_barrier = _drain_and_barrier
    tile.TileContext._ant_drain_patched = True


def _split_excess_waits(nc, mybir):
    """Walrus in this container accepts at most 1 sem-wait per instruction
    (2 for EventSemaphore). Tile can attach several. Move the extras onto
    fresh NoOps inserted just before the instruction on the same engine —
    same-engine streams execute in order, so semantics are preserved."""
    ctr = 0
    for fn in nc.m.functions:
        for bb in fn.blocks:
            out, changed = [], False
            for inst in bb.instructions:
                si = inst.sync_info
                waits = list(si.on_wait) if si and si.on_wait else []
                cap = 2 if isinstance(inst, mybir.InstEventSemaphore) else 1
                if len(waits) > cap:
                    for w in waits[:-cap]:
                        nop = mybir.InstNoOp(name=f"I-wsplit-{ctr}", ins=[], outs=[])
                        ctr += 1
                        nop.engine = inst.engine
                        nop.sync_info = mybir.SyncInfo(on_wait=[w], on_update=[])
                        out.append(nop)
                    si.on_wait = waits[-cap:]
                    changed = True
                out.append(inst)
            if changed:
                bb.instructions = out


def _build():
    import concourse.bass as bass
    import concourse.mybir as mybir
    import concourse.tile as tile

    _patch_tile_drain()
    f16 = mybir.dt.float16
    f32 = mybir.dt.float32

    nc = bass.Bass()
    bij = nc.declare_dram_parameter("bij", [P, T * PC], f16, isOutput=False)
    bc = nc.declare_dram_parameter("bc", [PC, IJ], f16, isOutput=False)
    xt = nc.declare_dram_parameter("xt", [P, T * B], f16, isOutput=False)
    ca_in = nc.declare_dram_parameter("ca", [B, PC], f32, isOutput=False)
    sa_in = nc.declare_dram_parameter("sa", [B, PC], f32, isOutput=False)
    # Output: 13 groups of 4 j-chunks, packed [128, 512] with batch rows at
    # partition offsets 0/32/64/96 (host unpacks). f16 halves store traffic.
    NG = (NJ + 3) // 4
    out = nc.declare_dram_parameter("out", [NG * P, 512], f16, isOutput=True)

    with tile.TileContext(nc) as tc:
        with (
            tc.tile_pool(name="big", bufs=1) as big,
            tc.tile_pool(name="small", bufs=1) as small,
            tc.tile_pool(name="ostage", bufs=6) as ostage,
            tc.tile_pool(name="ps_coef", bufs=1, space="PSUM") as ps_coef,
            tc.tile_pool(name="ps_out", bufs=6, space="PSUM") as ps_out,
        ):
            XT = big.tile([P, T * B], f16, tag="xt")
            BIJ = big.tile([P, T * PC], f16, tag="bij")
            BC = big.tile([PC, IJ], f16, tag="bc")
            CA = small.tile([B, PC], f32, tag="ca")
            SA = small.tile([B, PC], f32, tag="sa")

            # Input DMAs on SP's HWDGE ring: FIFO per engine, so program
            # order = HBM service order. GEMM1 inputs first.
            nc.sync.dma_start(CA[:], ca_in[:])
            nc.sync.dma_start(SA[:], sa_in[:])
            nc.sync.dma_start(XT[:], xt[:])
            nbij = 16
            for s in range(nbij):
                w = (T * PC) // nbij
                nc.sync.dma_start(BIJ[:, s * w : (s + 1) * w], bij[:, s * w : (s + 1) * w])
            nbc = 16
            for s in range(nbc):
                w = IJ // nbc
                nc.sync.dma_start(BC[:, s * w : (s + 1) * w], bc[:, s * w : (s + 1) * w])

            ROTP = small.tile([32, 128], f32, tag="rotp")
            nc.vector.memset(ROTP[:], 0.0)

            # GEMM1: coeffs[b, c] = sum_ij x[b, ij] * basis[c, ij]
            COEF = ps_coef.tile([B, PC], f32, tag="coef")
            for t in range(T):
                nc.tensor.matmul(
                    COEF[:],
                    XT[:, t * B : (t + 1) * B],
                    BIJ[:, t * PC : (t + 1) * PC],
                    start=(t == 0),
                    stop=(t == T - 1),
                )

            # Rotation: rot = coef*ca + swap_halves(coef)*sa  (sa sign-folded)
            TMP = small.tile([B, PC], f32, tag="tmp")
            TMP2 = small.tile([B, PC], f32, tag="tmp2")
            H = PC // 2
            nc.vector.tensor_mul(TMP[:], COEF[:], CA[:])
            nc.vector.tensor_mul(TMP2[:, 0:H], COEF[:, H:PC], SA[:, 0:H])
            nc.vector.tensor_mul(TMP2[:, H:PC], COEF[:, 0:H], SA[:, H:PC])
            nc.vector.tensor_add(ROTP[0:B, 0:PC], TMP[:], TMP2[:])

            # Transpose rot [8,100] -> rotT [100,8] via 32x32 DVE blocks
            ROTTP = small.tile([128, 32], f32, tag="rottp")
            for jb in range(4):
                nc.vector.transpose(
                    ROTTP[32 * jb : 32 * (jb + 1), 0:32],
                    ROTP[0:32, 32 * jb : 32 * (jb + 1)],
                )
            ROTT = small.tile([PC, B], f16, tag="rott")
            nc.vector.tensor_copy(ROTT[:], ROTTP[0:PC, 0:B])

            # GEMM2: out[b, ij] = sum_c rot[b, c] * basis[c, ij]
            # 4 j-chunks per PSUM bank at partition offsets 0/32/64/96 (PE
            # column-tiling; the 4 matmuls run concurrently in the array), so
            # evacuation is one full-width [128, 512] DVE copy per group
            # (with f32->f16 cast) instead of four 8-partition copies. Staged
            # output DMAs ride the ACT HWDGE ring to keep SP's input FIFO
            # unblocked; the host unpacks the group layout.
            for g in range(NG):
                js = list(range(g * 4, min((g + 1) * 4, NJ)))
                OPS = ps_out.tile([P, 512], f32, tag="ops")
                for i, j in enumerate(js):
                    nc.tensor.matmul(
                        OPS[32 * i : 32 * i + B, :],
                        ROTT[:],
                        BC[:, j * 512 : (j + 1) * 512],
                        start=True, stop=True,
                        tile_position=(0, 32 * i),
                    )
                OS = ostage.tile([P, 512], f16, tag="oc")
                hi = 32 * (len(js) - 1) + B
                nc.vector.tensor_copy(OS[0:hi, :], OPS[0:hi, :])
                nc.scalar.dma_start(out[g * P : g * P + hi, :], OS[0:hi, :])
    _split_excess_waits(nc, mybir)
    return nc


_CACHE = {}


def _get_nc():
    if "nc" not in _CACHE:
        _CACHE["nc"] = _build()
    return _CACHE["nc"]


def kernel(x, basis, angles):
    from concourse.bass_utils import run_bass_kernel_spmd

    x = np.asarray(x)
    basis = np.asarray(basis)
    angles = np.asarray(angles)

    X2 = x.reshape(B, IJ)
    BF = basis.reshape(NL2, IJ)
    xt16 = np.ascontiguousarray(X2.T).astype(np.float16).reshape(P, T * B)

    in_maps = []
    for k in range(NCORES):
        idx = np.concatenate(
            [np.arange(k * PC, (k + 1) * PC, 2), np.arange(k * PC + 1, (k + 1) * PC, 2)]
        )
        bc16 = BF[idx].astype(np.float16)                       # [100, 25600]
        bij16 = np.ascontiguousarray(
            bc16.reshape(PC, P, T).transpose(1, 2, 0)
        ).reshape(P, T * PC)                                    # [p, t*100+c]
        lvals = ((k * 50 + np.arange(50)) % 20).astype(np.float32)
        lrow = np.concatenate([lvals, -lvals])            # sign-folded l values
        theta = angles.astype(np.float32).reshape(B, 1) * lrow[None, :]
        ca = np.cos(theta).astype(np.float32)
        sa = np.sin(theta).astype(np.float32)             # second half = -sin
        in_maps.append({"bij": bij16, "bc": bc16, "xt": xt16, "ca": ca, "sa": sa})

    res = run_bass_kernel_spmd(_get_nc(), in_maps, list(range(NCORES)))
    NG = (NJ + 3) // 4
    stage = np.zeros((NG * P, 512), dtype=np.float32)
    for k in range(NCORES):
        stage += res.results[k]["out"].astype(np.float32)
    # unpack: row g*128 + 32*i + b, col n  ->  out[b, (4g+i)*512 + n]
    arr = stage.reshape(NG, 4, 32, 512)[:, :, 0:B, :]
    total = arr.transpose(2, 0, 1, 3).reshape(B, NG * 4 * 512)[:, :IJ]
    return np.ascontiguousarray(total).reshape(B, 1, 160, 160)



# revision 6
# speedup vs baseline: 1.3777x; 1.3777x over previous
"""Harmonic decomposition kernel for 8 TRN2 NeuronCores.

out[b] = basis^T R(theta_b) (basis @ x_b)   with per-harmonic complex rotation.

Sharding: the N*L*2 = 800 coefficient axis is split into 8 slices of 100 in
natural order (real parts on even rows, imaginary on odd rows). Each core
ships its basis slice ONCE in c-major layout (bc [100, 25600] f16), derives
the ij-major tiles needed by the projection on-chip (PE transpose -> PSUM f16
-> DVE evacuation), and runs both GEMMs with the basis slice as the
*stationary* matmul operand so each matmul only streams 8 batch columns:

  GEMM1 (projection):     coefT[c, b] += bijT_t[ij, c]^T @ xt_t[ij, b]
  rotate:                 rotT = coefT*ca + shuffle(coefT)*sa  (partition-pair
                          swap via stream_shuffle mask i^1; sa sign-folded)
  GEMM2 (reconstruction): outT_j[ij, b] = bc_j[c, ij]^T @ rotT[c, b]

A tunable tail of H ij-tiles is shipped pre-transposed from the host (bijh)
to balance PE-transpose cycles against DMA bandwidth and shorten the tail.
The host sums the 8 partial outputs.
"""

import sys

import numpy as np

for _p in ("/opt/trn_rl_repo",):
    if _p not in sys.path:
        sys.path.insert(0, _p)

B = 8          # batch
IJ = 25600     # 160*160 spatial
NL2 = 800      # total coefficients
PC = 100       # coefficients per core
P = 128        # partitions
NT = 200       # ij tiles (IJ / 128)
NCORES = 8

# --- tunables ---------------------------------------------------------------
H = 18                                  # hybrid tiles shipped pre-transposed
TR = NT - H                             # tiles transposed on-chip
CH_TR = [14] * 12 + [8, 4, 2]           # bc chunk sizes over t < TR (sum 182)
CH_G2 = [H]                             # bc chunk sizes over t >= TR
SSZ = 10                                # tiles per transpose stage (1 PSUM bank)
NSTG = 3                                # SBUF stage buffers
NPT = 3                                 # PSUM transpose buffers
GSZ = 64                                # GEMM2 j-tiles per PSUM bank
ACT_EVAC = ()                           # stage indices evacuated on ACT engine
assert sum(CH_TR) == TR and sum(CH_G2) == H


def _patch_tile_drain():
    """This container's walrus caps sem-waits at 1 per instruction; the stock
    Tile tail drain carries one wait per live semaphore. Keep one on the drain
    and emit the rest as individual SP wait instructions before the barrier."""
    import concourse.tile as tile
    from concourse.vector_clock import ScopedClock

    if getattr(tile.TileContext, "_ant_drain_patched", False):
        return

    def _drain_and_barrier(self, tick_clock, wait_clock):
        nc = self.nc
        drain_inst = nc.sync.drain()
        wait_clock.add_sem_waits(
            drain_inst.ins, ScopedClock({None: tick_clock.global_clock})
        )
        si = drain_inst.ins.sync_info
        waits = list(si.on_wait) if si and si.on_wait else []
        if len(waits) > 1:
            num2sem = {s.num: s for s in self.sems.allocated().values()}
            si.on_wait = waits[:1]
            for w in waits[1:]:
                op = {"sem-ge-imm": "sem-ge", "sem-eq-imm": "sem-eq"}[w.wait_mode]
                nc.sync.nop(nofuse=True).wait_op(num2sem[w.id], w.wait_value, op)
        nc.all_engine_barrier()
        assert self.sems is not None
        popped = nc._tile_sem_poison_stack.pop()
        assert popped is self._sem_poison
        nc.clear_and_free_semaphores(list(self.sems.allocated().values()))
        nc.all_engine_barrier()

    tile.TileContext._drain_and_barrier = _drain_and_barrier
    tile.TileContext._ant_drain_patched = True


def _split_excess_waits(nc, mybir):
    """Walrus in this container accepts at most 1 sem-wait per instruction
    (2 for EventSemaphore). Tile can attach several. Move the extras onto
    fresh NoOps inserted just before the instruction on the same engine —
    same-engine streams execute in order, so semantics are preserved."""
    ctr = 0
    for fn in nc.m.functions:
        for bb in fn.blocks:
            out, changed = [], False
            for inst in bb.instructions:
                si = inst.sync_info
                waits = list(si.on_wait) if si and si.on_wait else []
                cap = 2 if isinstance(inst, mybir.InstEventSemaphore) else 1
                if len(waits) > cap:
                    for w in waits[:-cap]:
                        nop = mybir.InstNoOp(name=f"I-wsplit-{ctr}", ins=[], outs=[])
                        ctr += 1
                        nop.engine = inst.engine
                        nop.sync_info = mybir.SyncInfo(on_wait=[w], on_update=[])
                        out.append(nop)
                    si.on_wait = waits[-cap:]
                    changed = True
                out.append(inst)
            if changed:
                bb.instructions = out


def _build():
    import concourse.bass as bass
    import concourse.mybir as mybir
    import concourse.tile as tile
    from concourse.masks import make_identity

    _patch_tile_drain()
    f16 = mybir.dt.float16
    f32 = mybir.dt.float32

    nc = bass.Bass()
    bc = nc.declare_dram_parameter("bc", [PC, IJ], f16, isOutput=False)
    xt = nc.declare_dram_parameter("xt", [P, NT * B], f16, isOutput=False)
    bijh = nc.declare_dram_parameter("bijh", [P, max(H, 1) * PC], f16, isOutput=False)
    casa = nc.declare_dram_parameter("casa", [PC, 2 * B], f32, isOutput=False)
    # outT tiles packed [ij_local 128, tile, b]: col 8*j + b = out_b[128*j + p]
    out = nc.declare_dram_parameter("out", [P, NT * B], f16, isOutput=True)

    # chunk start offsets (in tiles)
    chunks = []
    t0 = 0
    for n in CH_TR + CH_G2:
        chunks.append((t0, n))
        t0 += n
    n_tr_chunks = len(CH_TR)

    with tile.TileContext(nc) as tc:
        with (
            tc.tile_pool(name="big", bufs=1) as big,
            tc.tile_pool(name="small", bufs=1) as small,
            tc.tile_pool(name="stg", bufs=NSTG) as stg,
            tc.tile_pool(name="pt", bufs=NPT, space="PSUM") as ptp,
            tc.tile_pool(name="co", bufs=1, space="PSUM") as cop,
            tc.tile_pool(name="po", bufs=3, space="PSUM") as pop,
        ):
            BC = big.tile([PC, IJ], f16, tag="bc")
            XT = big.tile([P, NT * B], f16, tag="xt")
            BIJH = big.tile([P, max(H, 1) * PC], f16, tag="bijh")
            CASA = small.tile([PC, 2 * B], f32, tag="casa")
            IDENT = small.tile([PC, PC], f16, tag="ident")
            OST = big.tile([P, NT * B], f16, tag="ost")

            # identity for PE transpose, built on the otherwise-idle gpsimd
            make_identity(nc, IDENT[:])

            # Input DMAs on SP's HWDGE ring: FIFO per engine, so program
            # order = HBM service order.
            nc.sync.dma_start(CASA[:], casa[:])
            c0, n0 = chunks[0]
            nc.sync.dma_start(BC[:, c0 * P : (c0 + n0) * P], bc[:, c0 * P : (c0 + n0) * P])
            nc.sync.dma_start(XT[:], xt[:])
            for (ct, cn) in chunks[1:n_tr_chunks]:
                nc.sync.dma_start(BC[:, ct * P : (ct + cn) * P], bc[:, ct * P : (ct + cn) * P])
            if H > 0:
                nc.sync.dma_start(BIJH[:], bijh[:])
            for (ct, cn) in chunks[n_tr_chunks:]:
                nc.sync.dma_start(BC[:, ct * P : (ct + cn) * P], bc[:, ct * P : (ct + cn) * P])

            # coefT accumulator [128, 8] f32 (pad rows 100:128 zeroed so the
            # partition shuffle below never reads uninitialized PSUM)
            # zero the pad quadrant before the G1 group opens (start=True on
            # the first matmul re-resets rows 0:100, leaving 100:128 zero)
            CO = cop.tile([P, B], f32, tag="co")
            nc.vector.memset(CO[96:P, :], 0.0)

            # GEMM1 pipeline over transpose stages (decoupled from the DMA
            # chunking): PE transposes stage s into PSUM (f16), DVE evacuates
            # to an SBUF stage buffer, PE then consumes the stage as
            # *stationary* weights (8-col matmuls) one stage behind the
            # transposes so it never waits on the evac.
            stages = []
            t0 = 0
            while t0 < TR:
                n = min(SSZ, TR - t0)
                stages.append((t0, n))
                t0 += n
            stage_tiles = []   # (STG tile, stage) ready for G1
            g1_done = 0

            def g1_consume():
                nonlocal g1_done
                ST, (ct, cn) = stage_tiles.pop(0)
                for k in range(cn):
                    t = ct + k
                    nc.tensor.matmul(
                        CO[0:PC, :],
                        ST[:, k * PC : (k + 1) * PC],
                        XT[:, t * B : (t + 1) * B],
                        start=(g1_done == 0),
                        stop=False,
                    )
                    g1_done += 1

            for s, (ct, cn) in enumerate(stages):
                PT = ptp.tile([P, SSZ * PC], f16, tag="pt")
                for k in range(cn):
                    nc.tensor.transpose(
                        PT[:, k * PC : (k + 1) * PC],
                        BC[:, (ct + k) * P : (ct + k + 1) * P],
                        IDENT[:],
                    )
                ST = stg.tile([P, SSZ * PC], f16, tag="stg")
                if s in ACT_EVAC:
                    nc.scalar.copy(ST[:, 0 : cn * PC], PT[:, 0 : cn * PC])
                else:
                    nc.vector.tensor_copy(ST[:, 0 : cn * PC], PT[:, 0 : cn * PC])
                stage_tiles.append((ST, (ct, cn)))
                if len(stage_tiles) > 1:
                    g1_consume()
            while stage_tiles:
                g1_consume()

            # hybrid pre-transposed tiles close the accumulation group
            for i in range(H):
                t = TR + i
                nc.tensor.matmul(
                    CO[0:PC, :],
                    BIJH[:, i * PC : (i + 1) * PC],
                    XT[:, t * B : (t + 1) * B],
                    start=False,
                    stop=(i == H - 1),
                )

            # rotation: rotT = coefT*ca + swap_pairs(coefT)*sa (sa sign-folded)
            CSW = small.tile([P, B], f32, tag="csw")
            mask = [i ^ 1 for i in range(32)]
            nc.vector.stream_shuffle(CSW[:], CO[:], mask)
            TMP = small.tile([PC, B], f32, tag="tmp")
            TMP2 = small.tile([PC, B], f32, tag="tmp2")
            ROTT = small.tile([PC, B], f16, tag="rott")
            nc.vector.tensor_mul(TMP[:], CO[0:PC, :], CASA[:, 0:B])
            nc.vector.tensor_mul(TMP2[:], CSW[0:PC, :], CASA[:, B : 2 * B])
            nc.vector.tensor_add(ROTT[:], TMP[:], TMP2[:])

            # GEMM2: outT_j = bc_j^T @ rotT, 64 j-tiles per PSUM bank, ACT
            # evacuates each bank into the staged output, SP DMAs it out.
            ngroups = (NT + GSZ - 1) // GSZ
            odma = []
            for g in range(ngroups):
                js = range(g * GSZ, min((g + 1) * GSZ, NT))
                PO = pop.tile([P, GSZ * B], f32, tag="po")
                for i, j in enumerate(js):
                    nc.tensor.matmul(
                        PO[:, i * B : (i + 1) * B],
                        BC[:, j * P : (j + 1) * P],
                        ROTT[:],
                        start=True,
                        stop=True,
                    )
                w = len(js) * B
                nc.scalar.copy(OST[:, g * GSZ * B : g * GSZ * B + w], PO[:, 0:w])
                odma.append((g * GSZ * B, w))
            # merge output DMAs in pairs to halve HWDGE overhead
            i = 0
            while i < len(odma):
                o0, w0 = odma[i]
                if i + 1 < len(odma) and odma[i + 1][0] == o0 + w0:
                    w0 += odma[i + 1][1]
                    i += 2
                else:
                    i += 1
                nc.sync.dma_start(out[:, o0 : o0 + w0], OST[:, o0 : o0 + w0])
    _split_excess_waits(nc, mybir)
    return nc


_CACHE = {}


def _get_nc():
    if "nc" not in _CACHE:
        _CACHE["nc"] = _build()
    return _CACHE["nc"]


def prep_in_maps(x, basis, angles):
    x = np.asarray(x)
    basis = np.asarray(basis)
    angles = np.asarray(angles).astype(np.float32)

    X2 = x.reshape(B, IJ)
    BF = basis.reshape(NL2, IJ)
    xt16 = np.ascontiguousarray(
        X2.T.reshape(NT, P, B).transpose(1, 0, 2)
    ).reshape(P, NT * B).astype(np.float16)

    j = np.arange(PC)
    sign = np.where(j % 2 == 0, 1.0, -1.0).astype(np.float32)

    in_maps = []
    for k in range(NCORES):
        bc16 = BF[k * PC : (k + 1) * PC].astype(np.float16)     # natural order
        if H > 0:
            bijh = np.ascontiguousarray(
                bc16[:, TR * P :].reshape(PC, H, P).transpose(2, 1, 0)
            ).reshape(P, H * PC)
        else:
            bijh = np.zeros((P, PC), dtype=np.float16)
        lvals = ((k * 50 + j // 2) % 20).astype(np.float32)
        theta = lvals[:, None] * angles[None, :]                # [100, 8]
        casa = np.concatenate(
            [np.cos(theta), np.sin(theta) * sign[:, None]], axis=1
        ).astype(np.float32)                                    # [100, 16]
        in_maps.append({"bc": bc16, "xt": xt16, "bijh": bijh, "casa": casa})
    return in_maps


def kernel(x, basis, angles):
    from concourse.bass_utils import run_bass_kernel_spmd

    in_maps = prep_in_maps(x, basis, angles)
    res = run_bass_kernel_spmd(_get_nc(), in_maps, list(range(NCORES)))
    stage = np.zeros((P, NT * B), dtype=np.float32)
    for k in range(NCORES):
        stage += res.results[k]["out"].astype(np.float32)
    # col 8*j + b, row p  ->  out[b, 128*j + p]
    total = stage.reshape(P, NT, B).transpose(2, 1, 0).reshape(B, IJ)
    return np.ascontiguousarray(total).reshape(B, 1, 160, 160)


# revision 9
# speedup vs baseline: 1.3984x; 1.0150x over previous
"""Harmonic decomposition kernel for 8 TRN2 NeuronCores.

out[b] = basis^T R(theta_b) (basis @ x_b)   with per-harmonic complex rotation.

Sharding: the N*L*2 = 800 coefficient axis is split into 8 slices of 100 in
natural order (real parts on even rows, imaginary on odd rows). Each core
ships its basis slice ONCE in c-major layout (bc [100, 25600] f16), derives
the ij-major tiles needed by the projection on-chip (PE transpose -> PSUM f16
-> DVE/ACT evacuation), and runs both GEMMs with the basis slice as the
*stationary* matmul operand so each matmul only streams 8 batch columns:

  GEMM1 (projection):     coefT[c, b] += bijT_t[ij, c]^T @ xt_t[ij, b]
  rotate:                 rotT = coefT*ca + shuffle(coefT)*sa  (partition-pair
                          swap via stream_shuffle mask i^1; sa sign-folded)
  GEMM2 (reconstruction): outT_j[ij, b] = bc_j[c, ij]^T @ rotT[c, b]

A tunable tail of H ij-tiles is shipped pre-transposed from the host (bijh)
to balance PE-transpose cycles against DMA bandwidth and shorten the tail.
GEMM2 results are DMA'd straight from PSUM as f32 partials (no staging hop);
the host sums the 8 partials.
"""

import sys

import numpy as np

for _p in ("/opt/trn_rl_repo",):
    if _p not in sys.path:
        sys.path.insert(0, _p)

B = 8          # batch
IJ = 25600     # 160*160 spatial
NL2 = 800     # total coefficients
PC = 100       # coefficients per core
P = 128        # partitions
NT = 200       # ij tiles (IJ / 128)
NCORES = 8

# --- tunables ---------------------------------------------------------------
H = 18                                  # hybrid tiles shipped pre-transposed
TR = NT - H                             # tiles transposed on-chip
CH_TR = [6] + [20] * 8 + [16]           # bc chunk sizes over t < TR (sum 182)
CH_G2 = [H]                             # bc chunk sizes over t >= TR
STAGES = [6] + [10] * 17 + [6]          # transpose stage sizes (chunk-aligned)
G1_LAG = 2                              # stages between transpose and G1 use
NSTG = 5                                # SBUF stage buffers
NPT = 3                                 # PSUM transpose buffers
GSZ = 64                                # GEMM2 j-tiles per PSUM bank
NPO = 4                                 # GEMM2 PSUM banks
XT_AFTER_CHUNK = 1                      # issue xt DMA after this many chunks
assert sum(CH_TR) == TR and sum(CH_G2) == H and sum(STAGES) == TR


def _patch_tile_drain():
    """This container's walrus caps sem-waits at 1 per instruction; the stock
    Tile tail drain carries one wait per live semaphore. Keep one on the drain
    and emit the rest as individual SP wait instructions before the barrier."""
    import concourse.tile as tile
    from concourse.vector_clock import ScopedClock

    if getattr(tile.TileContext, "_ant_drain_patched", False):
        return

    def _drain_and_barrier(self, tick_clock, wait_clock):
        nc = self.nc
        drain_inst = nc.sync.drain()
        wait_clock.add_sem_waits(
            drain_inst.ins, ScopedClock({None: tick_clock.global_clock})
        )
        si = drain_inst.ins.sync_info
        waits = list(si.on_wait) if si and si.on_wait else []
        if len(waits) > 1:
            num2sem = {s.num: s for s in self.sems.allocated().values()}
            si.on_wait = waits[:1]
            for w in waits[1:]:
                op = {"sem-ge-imm": "sem-ge", "sem-eq-imm": "sem-eq"}[w.wait_mode]
                nc.sync.nop(nofuse=True).wait_op(num2sem[w.id], w.wait_value, op)
        nc.all_engine_barrier()
        assert self.sems is not None
        popped = nc._tile_sem_poison_stack.pop()
        assert popped is self._sem_poison
        nc.clear_and_free_semaphores(list(self.sems.allocated().values()))
        nc.all_engine_barrier()

    tile.TileContext._drain_and_barrier = _drain_and_barrier
    tile.TileContext._ant_drain_patched = True


def _split_excess_waits(nc, mybir):
    """Walrus in this container accepts at most 1 sem-wait per instruction
    (2 for EventSemaphore). Tile can attach several. Move the extras onto
    fresh NoOps inserted just before the instruction on the same engine —
    same-engine streams execute in order, so semantics are preserved."""
    ctr = 0
    for fn in nc.m.functions:
        for bb in fn.blocks:
            out, changed = [], False
            for inst in bb.instructions:
                si = inst.sync_info
                waits = list(si.on_wait) if si and si.on_wait else []
                cap = 2 if isinstance(inst, mybir.InstEventSemaphore) else 1
                if len(waits) > cap:
                    for w in waits[:-cap]:
                        nop = mybir.InstNoOp(name=f"I-wsplit-{ctr}", ins=[], outs=[])
                        ctr += 1
                        nop.engine = inst.engine
                        nop.sync_info = mybir.SyncInfo(on_wait=[w], on_update=[])
                        out.append(nop)
                    si.on_wait = waits[-cap:]
                    changed = True
                out.append(inst)
            if changed:
                bb.instructions = out


def _build():
    import concourse.bass as bass
    import concourse.mybir as mybir
    import concourse.tile as tile
    from concourse.masks import make_identity

    _patch_tile_drain()
    f16 = mybir.dt.float16
    f32 = mybir.dt.float32

    nc = bass.Bass()
    bc = nc.declare_dram_parameter("bc", [PC, IJ], f16, isOutput=False)
    xt = nc.declare_dram_parameter("xt", [P, NT * B], f16, isOutput=False)
    bijh = nc.declare_dram_parameter("bijh", [P, max(H, 1) * PC], f16, isOutput=False)
    casa = nc.declare_dram_parameter("casa", [PC, 2 * B], f32, isOutput=False)
    # outT tiles packed [ij_local 128, tile, b]: col 8*j + b = out_b[128*j + p]
    out = nc.declare_dram_parameter("out", [P, NT * B], f16, isOutput=True)

    with tile.TileContext(nc) as tc:
        with (
            tc.tile_pool(name="big", bufs=1) as big,
            tc.tile_pool(name="small", bufs=1) as small,
            tc.tile_pool(name="stg", bufs=NSTG) as stg,
            tc.tile_pool(name="pt", bufs=NPT, space="PSUM") as ptp,
            tc.tile_pool(name="co", bufs=1, space="PSUM") as cop,
            tc.tile_pool(name="po", bufs=NPO, space="PSUM") as pop,
        ):
            BC = big.tile([PC, IJ], f16, tag="bc")
            XT = big.tile([P, NT * B], f16, tag="xt")
            BIJH = big.tile([P, max(H, 1) * PC], f16, tag="bijh")
            CASA = small.tile([PC, 2 * B], f32, tag="casa")
            IDENT = small.tile([PC, PC], f16, tag="ident")

            # identity for PE transpose, built on the otherwise-idle gpsimd
            make_identity(nc, IDENT[:])

            # Input DMAs: casa rides the ACT ring so SP's first bc chunk isn't
            # delayed; everything else is FIFO on SP's ring in arrival order.
            nc.scalar.dma_start(CASA[:], casa[:])
            t0 = 0
            for i, cn in enumerate(CH_TR):
                nc.sync.dma_start(BC[:, t0 * P : (t0 + cn) * P], bc[:, t0 * P : (t0 + cn) * P])
                t0 += cn
                if i + 1 == XT_AFTER_CHUNK:
                    nc.sync.dma_start(XT[:], xt[:])
            if H > 0:
                nc.sync.dma_start(BIJH[:], bijh[:])
            for cn in CH_G2:
                nc.sync.dma_start(BC[:, t0 * P : (t0 + cn) * P], bc[:, t0 * P : (t0 + cn) * P])
                t0 += cn

            # coefT accumulator [128, 8] f32; zero the pad quadrant before the
            # G1 group opens (start=True re-resets rows 0:100, leaving 100:128
            # zero for the partition shuffle below)
            CO = cop.tile([P, B], f32, tag="co")
            nc.vector.memset(CO[96:P, :], 0.0)

            # GEMM1 pipeline: PE transposes stage s into PSUM (f16), DVE/ACT
            # evacuate to an SBUF stage buffer, PE consumes the stage as
            # *stationary* weights (8-col matmuls) G1_LAG stages behind the
            # transposes so the PE->evac->PE round trip stays off the PE's
            # critical path.
            stages = []
            t0 = 0
            for n in STAGES:
                stages.append((t0, n))
                t0 += n
            stage_tiles = []   # (STG tile, stage) ready for G1
            g1_done = 0

            def g1_consume():
                nonlocal g1_done
                ST, (ct, cn) = stage_tiles.pop(0)
                for k in range(cn):
                    t = ct + k
                    nc.tensor.matmul(
                        CO[0:PC, :],
                        ST[:, k * PC : (k + 1) * PC],
                        XT[:, t * B : (t + 1) * B],
                        start=(g1_done == 0),
                        stop=False,
                    )
                    g1_done += 1

            for s, (ct, cn) in enumerate(stages):
                PT = ptp.tile([P, 10 * PC], f16, tag="pt")
                for k in range(cn):
                    nc.tensor.transpose(
                        PT[:, k * PC : (k + 1) * PC],
                        BC[:, (ct + k) * P : (ct + k + 1) * P],
                        IDENT[:],
                    )
                ST = stg.tile([P, 10 * PC], f16, tag="stg")
                if s % 2 == 1:
                    nc.scalar.copy(ST[:, 0 : cn * PC], PT[:, 0 : cn * PC])
                else:
                    nc.vector.tensor_copy(ST[:, 0 : cn * PC], PT[:, 0 : cn * PC])
                stage_tiles.append((ST, (ct, cn)))
                if len(stage_tiles) > G1_LAG:
                    g1_consume()
            while stage_tiles:
                g1_consume()

            # hybrid pre-transposed tiles close the accumulation group
            for i in range(H):
                t = TR + i
                nc.tensor.matmul(
                    CO[0:PC, :],
                    BIJH[:, i * PC : (i + 1) * PC],
                    XT[:, t * B : (t + 1) * B],
                    start=False,
                    stop=(i == H - 1),
                )

            # rotation: rotT = coefT*ca + swap_pairs(coefT)*sa (sa sign-folded)
            CSW = small.tile([P, B], f32, tag="csw")
            mask = [i ^ 1 for i in range(32)]
            nc.vector.stream_shuffle(CSW[:], CO[:], mask)
            TMP = small.tile([PC, B], f32, tag="tmp")
            TMP2 = small.tile([PC, B], f32, tag="tmp2")
            ROTT = small.tile([PC, B], f16, tag="rott")
            nc.vector.tensor_mul(TMP[:], CO[0:PC, :], CASA[:, 0:B])
            nc.vector.tensor_mul(TMP2[:], CSW[0:PC, :], CASA[:, B : 2 * B])
            nc.vector.tensor_add(ROTT[:], TMP[:], TMP2[:])

            # GEMM2: outT_j = bc_j^T @ rotT, 64 j-tiles per PSUM bank.
            # Banks evacuate to SBUF (f32->f16) on alternating engines and
            # each is DMA'd out as soon as it lands (4 pipelined SP DMAs).
            OST = big.tile([P, NT * B], f16, tag="ost")
            ngroups = (NT + GSZ - 1) // GSZ
            for g in range(ngroups):
                js = range(g * GSZ, min((g + 1) * GSZ, NT))
                PO = pop.tile([P, GSZ * B], f32, tag="po")
                for i, j in enumerate(js):
                    nc.tensor.matmul(
                        PO[:, i * B : (i + 1) * B],
                        BC[:, j * P : (j + 1) * P],
                        ROTT[:],
                        start=True,
                        stop=True,
                    )
                w = len(js) * B
                o0 = g * GSZ * B
                if g % 2 == 0:
                    nc.scalar.copy(OST[:, o0 : o0 + w], PO[:, 0:w])
                else:
                    nc.vector.tensor_copy(OST[:, o0 : o0 + w], PO[:, 0:w])
                nc.sync.dma_start(out[:, o0 : o0 + w], OST[:, o0 : o0 + w])
    _split_excess_waits(nc, mybir)
    return nc


_CACHE = {}


def _get_nc():
    if "nc" not in _CACHE:
        _CACHE["nc"] = _build()
    return _CACHE["nc"]


def prep_in_maps(x, basis, angles):
    x = np.asarray(x)
    basis = np.asarray(basis)
    angles = np.asarray(angles).astype(np.float32)

    X2 = x.reshape(B, IJ)
    BF = basis.reshape(NL2, IJ)
    xt16 = np.ascontiguousarray(
        X2.T.reshape(NT, P, B).transpose(1, 0, 2)
    ).reshape(P, NT * B).astype(np.float16)

    j = np.arange(PC)
    sign = np.where(j % 2 == 0, 1.0, -1.0).astype(np.float32)

    in_maps = []
    for k in range(NCORES):
        bc16 = BF[k * PC : (k + 1) * PC].astype(np.float16)     # natural order
        if H > 0:
            bijh = np.ascontiguousarray(
                bc16[:, TR * P :].reshape(PC, H, P).transpose(2, 1, 0)
            ).reshape(P, H * PC)
        else:
            bijh = np.zeros((P, PC), dtype=np.float16)
        lvals = ((k * 50 + j // 2) % 20).astype(np.float32)
        theta = lvals[:, None] * angles[None, :]                # [100, 8]
        casa = np.concatenate(
            [np.cos(theta), np.sin(theta) * sign[:, None]], axis=1
        ).astype(np.float32)                                    # [100, 16]
        in_maps.append({"bc": bc16, "xt": xt16, "bijh": bijh, "casa": casa})
    return in_maps


def kernel(x, basis, angles):
    from concourse.bass_utils import run_bass_kernel_spmd

    in_maps = prep_in_maps(x, basis, angles)
    res = run_bass_kernel_spmd(_get_nc(), in_maps, list(range(NCORES)))
    stage = np.zeros((P, NT * B), dtype=np.float32)
    for k in range(NCORES):
        stage += res.results[k]["out"].astype(np.float32)
    # col 8*j + b, row p  ->  out[b, 128*j + p]
    total = stage.reshape(P, NT, B).transpose(2, 1, 0).reshape(B, IJ)
    return np.ascontiguousarray(total).reshape(B, 1, 160, 160)
